# revision 1
# baseline (speedup 1.0000x reference)
"""Trainium2 Bass kernel for nn_CategorySpecificInitNet (moe_routing).

kernel(**inputs) takes the FULL unsharded inputs (keys as in
reference.setup_inputs()) and returns the FULL [B, 128] float32 output.

Strategy — expert-parallel, per the spec sharding hint's dispatch-by-category
alternative:
  - the host sharding layer dispatches rows to cores by category (the
    "all-to-all dispatch by category" of expert-parallel, realized where
    all sharding happens in this harness): rows are stably sorted by
    cat_idx and core k receives category k's rows, zero-padded to a
    static per-core capacity (max category count rounded up to the
    512-row tile size);
  - every core runs the shared encoder plus exactly ONE decoder (its
    category's), so no routing, masking, or gather happens per row —
    the decoder FLOPs drop 8x vs computing all decoders densely;
  - the encoder's linear third layer is constant-folded into the
    decoder's first layer on the host (W_f = We3 @ Wd1_k,
    b_f = Wd1_k^T be3 + bd1_k — exact algebra, ~0.1% of the FLOPs),
    removing one full matmul stage from the device;
  - all compute is feature-major [features(partitions), rows(free)], so
    no transposes are ever needed on device (the host passes features
    pre-transposed); outputs come back [128, cap] and the host
    inverse-permutes rows during unsharding.
  - per-core row tiles of 512; the decoder stages are software-pipelined
    one tile behind the encoder so the PE never waits on ACT/DVE
    relu latency.

Matmuls run in float32r (fp32 storage, full PE rate at N=512, ~tf32-grade
multiply precision on HW — measured ~3e-4 max rel error vs the fp32
reference, 17x better than bf16 at the same PE throughput).
"""
import sys

for _p in ("/opt/trn_rl_repo",):
    if _p not in sys.path:
        sys.path.append(_p)

import numpy as np

import concourse.bass as bass
import concourse.bacc as bacc
import concourse.mybir as mybir
import concourse.tile as tile
from concourse import bass_utils

FR = mybir.dt.float32r
F32 = mybir.dt.float32
Alu = mybir.AluOpType
ActF = mybir.ActivationFunctionType

B, C, H1, H2, HO = 32768, 768, 512, 256, 256
DH, LAT, K = 256, 128, 8
N_CORES = 8
TILE = 512
# bias_all columns: be1[4] be2[2] bf[2](=Wd1^T be3 + bd1) bd2[2] bd3[1]
OB1, OB2, OD1, OD2, OD3 = 0, 4, 6, 8, 10
NBIAS = 11


def _build_nc(cap, tile_n=512, ps_w_bufs=6, dp_bufs=2, ps_o_bufs=2, split=(3, 3), tail_pos=1, ap_bufs=3, fp_bufs=2):
    assert cap % 256 == 0
    tiles = [tile_n] * (cap // tile_n)
    if cap % tile_n:
        tiles.insert(tail_pos if tail_pos is not None else len(tiles),
                     cap % tile_n)
    offs = [sum(tiles[:i]) for i in range(len(tiles))]
    nt = len(tiles)
    nc = bacc.Bacc(name="catnet_ep")

    fT = nc.dram_tensor("fT", (C, cap), FR, kind="ExternalInput")
    we1 = nc.dram_tensor("we1", (C, H1), FR, kind="ExternalInput")
    we2 = nc.dram_tensor("we2", (H1, H2), FR, kind="ExternalInput")
    wd1 = nc.dram_tensor("wd1", (H2, DH), FR, kind="ExternalInput")  # We3 @ Wd1
    wd2 = nc.dram_tensor("wd2", (DH, DH), FR, kind="ExternalInput")
    wd3 = nc.dram_tensor("wd3", (DH, LAT), FR, kind="ExternalInput")
    bias_all = nc.dram_tensor("bias_all", (128, NBIAS), F32, kind="ExternalInput")
    out = nc.dram_tensor("out", (LAT, cap), F32, kind="ExternalOutput")

    nC, nH1, nH2, nHO, nDH = C // 128, H1 // 128, H2 // 128, HO // 128, DH // 128

    with tile.TileContext(nc) as tc:
        with (
            tc.tile_pool(name="wp", bufs=1) as wp,
            tc.tile_pool(name="fp", bufs=fp_bufs) as fp,
            tc.tile_pool(name="ap", bufs=ap_bufs) as ap,
            tc.tile_pool(name="dp", bufs=dp_bufs) as dp,
            tc.tile_pool(name="ps_w", bufs=ps_w_bufs, space="PSUM") as ps_w,
            tc.tile_pool(name="ps_o", bufs=ps_o_bufs, space="PSUM") as ps_o,
        ):
            # we1 first, in two half-tensor DMAs: per-HWDGE-DMA queue issue
            # costs ~0.6us, so 2 DMAs beats 6 for total latency while still
            # letting the first L1 matmuls start after the first half lands
            we1_t = wp.tile([128, nC, H1], FR, tag="we1")
            we1_r = we1.rearrange("(c p) h -> p c h", p=128)
            c0 = 0
            for w in split:
                nc.gpsimd.dma_start(we1_t[:, c0:c0 + w, :], we1_r[:, c0:c0 + w, :])
                c0 += w
            bias_t = wp.tile([128, NBIAS], F32, tag="bias")
            nc.gpsimd.dma_start(bias_t[:], bias_all[:])
            we2_t = wp.tile([128, nH1, H2], FR, tag="we2")
            nc.gpsimd.dma_start(we2_t[:], we2.rearrange("(c p) h -> p c h", p=128))
            wd1_t = wp.tile([128, nH2, DH], FR, tag="wd1")
            nc.gpsimd.dma_start(wd1_t[:], wd1.rearrange("(c p) d -> p c d", p=128))
            wd2_t = wp.tile([128, nDH, DH], FR, tag="wd2")
            nc.gpsimd.dma_start(wd2_t[:], wd2.rearrange("(c p) d -> p c d", p=128))
            wd3_t = wp.tile([128, nDH, LAT], FR, tag="wd3")
            nc.gpsimd.dma_start(wd3_t[:], wd3.rearrange("(c p) d -> p c d", p=128))

            def emit_enc(t):
                tn = tiles[t]
                sl = slice(offs[t], offs[t] + tn)
                ftb_fl = fp.tile([128, nC, tile_n], FR, tag="ft")
                ftb = ftb_fl[:, :, :tn]
                if t == 0:
                    fr = fT.rearrange("(c p) b -> p c b", p=128)[:, :, sl]
                    c0 = 0
                    for w in split:
                        nc.sync.dma_start(ftb[:, c0:c0 + w, :], fr[:, c0:c0 + w, :])
                        c0 += w
                else:
                    # alternate queues so consecutive feature tiles stream in
                    # parallel instead of serializing on one HWDGE queue
                    eng = nc.sync if t % 2 == 0 else nc.gpsimd
                    eng.dma_start(
                        ftb[:], fT.rearrange("(c p) b -> p c b", p=128)[:, :, sl])
                pwsl = slice(0, tn)
                a1 = []
                for m in range(nH1):
                    pw_fl = ps_w.tile([128, tile_n], F32, tag="pw")
                    pw = pw_fl[:, :tn]
                    for c in range(nC):
                        nc.tensor.matmul(pw[:], we1_t[:, c, bass.ts(m, 128)],
                                         ftb[:, c, :],
                                         start=(c == 0), stop=(c == nC - 1))
                    x_fl = ap.tile([128, tile_n], FR, tag=f"a1_{m}")
                    x = x_fl[:, :tn]
                    nc.scalar.activation(x[:], pw[:], ActF.Relu,
                                         bias=bias_t[:, OB1 + m:OB1 + m + 1])
                    a1.append(x)
                a2 = []
                for m in range(nH2):
                    pw_fl = ps_w.tile([128, tile_n], F32, tag="pw")
                    pw = pw_fl[:, :tn]
                    for c in range(nH1):
                        nc.tensor.matmul(pw[:], we2_t[:, c, bass.ts(m, 128)], a1[c][:],
                                         start=(c == 0), stop=(c == nH1 - 1))
                    x_fl = ap.tile([128, tile_n], FR, tag=f"a2_{m}")
                    x = x_fl[:, :tn]
                    if m % 2 == 0:
                        nc.vector.tensor_scalar(x[:], pw[:],
                                                bias_t[:, OB2 + m:OB2 + m + 1],
                                                0.0, Alu.add, Alu.max)
                    else:
                        nc.scalar.activation(x[:], pw[:], ActF.Relu,
                                             bias=bias_t[:, OB2 + m:OB2 + m + 1])
                    a2.append(x)
                return a2

            def emit_d1(t, h):
                tn = tiles[t]
                d1 = []
                for m in range(nDH):
                    pw_fl = ps_w.tile([128, tile_n], F32, tag="pw")
                    pw = pw_fl[:, :tn]
                    for c in range(nH2):
                        nc.tensor.matmul(pw[:], wd1_t[:, c, bass.ts(m, 128)], h[c][:],
                                         start=(c == 0), stop=(c == nH2 - 1))
                    x_fl = dp.tile([128, tile_n], FR, tag=f"d1_{m}")
                    x = x_fl[:, :tn]
                    if m % 2 == 1:
                        nc.vector.tensor_scalar(x[:], pw[:],
                                                bias_t[:, OD1 + m:OD1 + m + 1],
                                                0.0, Alu.add, Alu.max)
                    else:
                        nc.scalar.activation(x[:], pw[:], ActF.Relu,
                                             bias=bias_t[:, OD1 + m:OD1 + m + 1])
                    d1.append(x)
                return d1

            def emit_d2_d3_store(t, d1):
                tn = tiles[t]
                d2 = []
                for m in range(nDH):
                    pw_fl = ps_w.tile([128, tile_n], F32, tag="pw")
                    pw = pw_fl[:, :tn]
                    for c in range(nDH):
                        nc.tensor.matmul(pw[:], wd2_t[:, c, bass.ts(m, 128)], d1[c][:],
                                         start=(c == 0), stop=(c == nDH - 1))
                    x_fl = dp.tile([128, tile_n], FR, tag=f"d2_{m}")
                    x = x_fl[:, :tn]
                    bb = bias_t[:, OD2 + m:OD2 + m + 1]
                    if m % 2 == 0:
                        nc.vector.tensor_scalar(x[:], pw[:], bb, 0.0, Alu.add, Alu.max)
                    else:
                        nc.scalar.activation(x[:], pw[:], ActF.Relu, bias=bb)
                    d2.append(x)
                po_fl = ps_o.tile([128, tile_n], F32, tag="out")
                po = po_fl[:, :tn]
                for c in range(nDH):
                    nc.tensor.matmul(po[:], wd3_t[:, c, :], d2[c][:],
                                     start=(c == 0), stop=(c == nDH - 1))
                osb_fl = ap.tile([128, tile_n], F32, tag="osb")
                osb = osb_fl[:, :tn]
                nc.scalar.activation(osb[:], po[:], ActF.Identity,
                                     bias=bias_t[:, OD3:OD3 + 1])
                nc.gpsimd.dma_start(out[:, offs[t]:offs[t] + tn], osb[:])

            # decoder runs one tile behind the encoder: PE order per step is
            # [enc t][d2/d3 t-1][d1 t], hiding ACT/DVE relu latency behind
            # independent matmuls
            pend = None
            for t in range(nt):
                h = emit_enc(t)
                if pend is not None:
                    emit_d2_d3_store(pend[0], pend[1])
                d1 = emit_d1(t, h)
                pend = (t, d1)
            emit_d2_d3_store(pend[0], pend[1])

    nc.finalize()
    return nc


def _pack_inputs(features, We1, be1, We2, be2, We3, be3,
                 Wd1, bd1, Wd2, bd2, Wd3, bd3, cat_idx, cap):
    """Dispatch rows to cores by category (expert-parallel sharding)."""
    features = np.asarray(features, np.float32)
    cat = np.asarray(cat_idx).astype(np.int64)
    order = np.argsort(cat, kind="stable")
    counts = np.bincount(cat, minlength=N_CORES)
    starts = np.zeros(N_CORES + 1, np.int64)
    np.cumsum(counts, out=starts[1:])

    def chunkcols(b):
        b = np.asarray(b, np.float32).reshape(-1)
        return b.reshape(-1, 128).T

    enc = dict(
        we1=np.asarray(We1, np.float32), we2=np.asarray(We2, np.float32),
    )
    We3f = np.asarray(We3, np.float32)
    be3f = np.asarray(be3, np.float32)
    maps, rows_per_core = [], []
    for k in range(N_CORES):
        rows = order[starts[k]:starts[k + 1]]
        rows_per_core.append(rows)
        f = np.zeros((cap, C), np.float32)
        f[:len(rows)] = features[rows]
        bias_all = np.zeros((128, NBIAS), np.float32)
        bias_all[:, OB1:OB1 + 4] = chunkcols(be1)
        bias_all[:, OB2:OB2 + 2] = chunkcols(be2)
        wd1k = np.asarray(Wd1, np.float32)[k]
        bias_all[:, OD1:OD1 + 2] = chunkcols(
            wd1k.T @ be3f + np.asarray(bd1, np.float32)[k])
        bias_all[:, OD2:OD2 + 2] = chunkcols(np.asarray(bd2, np.float32)[k])
        bias_all[:, OD3:OD3 + 1] = chunkcols(np.asarray(bd3, np.float32)[k])
        m = dict(enc)
        m["fT"] = np.ascontiguousarray(f.T)
        m["wd1"] = We3f @ wd1k  # encoder L3 folded into decoder layer 1
        m["wd2"] = np.asarray(Wd2, np.float32)[k]
        m["wd3"] = np.asarray(Wd3, np.float32)[k]
        m["bias_all"] = bias_all
        maps.append(m)
    return maps, rows_per_core


_NC_CACHE = {}


def _get_nc(cap=4352):
    if cap not in _NC_CACHE:
        _NC_CACHE[cap] = _build_nc(cap)
    return _NC_CACHE[cap]


def kernel(**inputs) -> np.ndarray:
    cat = np.asarray(inputs["cat_idx"]).astype(np.int64)
    counts = np.bincount(cat, minlength=N_CORES)
    cap = max(256, int(-(-counts.max() // 256) * 256))
    maps, rows_per_core = _pack_inputs(**inputs, cap=cap)
    nc = _get_nc(cap)
    res = bass_utils.run_bass_kernel_spmd(nc, maps, core_ids=list(range(N_CORES)))
    latent = np.zeros((B, LAT), np.float32)
    for k, r in enumerate(res.results):
        rows = rows_per_core[k]
        latent[rows] = r["out"][:, :len(rows)].T
    return latent



# revision 5
# speedup vs baseline: 1.1646x; 1.1646x over previous
"""Trainium2 Bass kernel for nn_CategorySpecificInitNet (moe_routing).

kernel(**inputs) takes the FULL unsharded inputs (keys as in
reference.setup_inputs()) and returns the FULL [B, 128] float32 output.

Strategy — expert-parallel (dispatch-by-category): rows are stably sorted
by cat_idx, core k gets category k's rows (zero-padded to the max
category count), so each core runs the shared encoder plus exactly ONE
decoder.  On top of the fp32r baseline:
  - L1 (768->512, 57% of PE work) runs as THREE-TERM hi/lo fp8-e4m3
    DoubleRow matmuls: x*16 = xh + xl (host split), W*256 = Wh + Wl
    (host split); psum = xh@Wh + xl@Wh + xh@Wl (the dropped lo@lo term
    is ~6e-4 relative).  DoubleRow contracts TWO 128-chunks per
    instruction at 0.5 cycles/row, so L1 costs 0.75x its fp32r cost at
    ~1.3e-3 final relative error (plain fp8 would be 4e-2).  All three
    terms pair adjacent feature chunks (c, c+1), so the stationaries are
    c-paired hi and lo weight stacks and the moving APs slice the
    interleaved [c][hi/lo][rows] feature layout.
  - L2/D1/D2/D3 run in fp16 (same PE rate as fp32r at any N, half the
    DMA bytes, ~4x better precision than bf16); the output rides back
    fp16 and is upcast on host.
  - encoder L3 is constant-folded into the decoder's first layer
    (W = We3 @ Wd1_k, b = Wd1_k^T be3 + bd1_k).
  - a warmup matmul chain (on a memset scratch tile) keeps the PE busy
    from ~1us so the p-state ramp finishes during the startup DMA window.
  - all loads ride ONE hand-ordered sync HWDGE queue (weights appear
    exactly before their first use deadline; the DMA engine pool is a
    serial resource so order == arrival); stores ride the scalar HWDGE
    queue right behind the producing activation.
  - per-core capacity is the exact max category count; the odd-size tail
    tile is processed SECOND so the end of the pipeline is fat 512-row
    tiles and the final act+store drain is the only exposed latency.
"""
import sys

for _p in ("/opt/trn_rl_repo",):
    if _p not in sys.path:
        sys.path.append(_p)

import numpy as np
import ml_dtypes

import concourse.bass as bass
import concourse.bacc as bacc
import concourse.mybir as mybir
import concourse.tile as tile
from concourse import bass_utils

FP8 = ml_dtypes.float8_e4m3
F8 = mybir.dt.float8e4
F16 = mybir.dt.float16
F32 = mybir.dt.float32
DR = mybir.MatmulPerfMode.DoubleRow
Alu = mybir.AluOpType
ActF = mybir.ActivationFunctionType

B, C, H1, H2, HO = 32768, 768, 512, 256, 256
DH, LAT, K = 256, 128, 8
N_CORES = 8
TILE = 512
SX, SW = 16.0, 256.0
L1SC = float(1.0 / (SX * SW))
# bias_all columns: be1[4] be2[2] bf[2](=Wd1^T be3 + bd1) bd2[2] bd3[1]
OB1, OB2, OD1, OD2, OD3 = 0, 4, 6, 8, 10
NBIAS = 11

nC, nH1, nH2, nDH = C // 128, H1 // 128, H2 // 128, DH // 128
nP = nC // 2  # feature-chunk pairs


def _build_nc(cap, n_warm=14, warm_n=256):
    # tail tile processed SECOND so the drain runs on fat 512-row tiles
    tiles = [TILE]
    if cap % TILE:
        tiles.append(cap % TILE)
    tiles += [TILE] * (cap // TILE - 1)
    offs = [sum(tiles[:i]) for i in range(len(tiles))]
    nt = len(tiles)
    nc = bacc.Bacc(name="catnet_fp8")

    # host-packed, partition-major dram layouts
    fTp = nc.dram_tensor("fTp", (128, nC, 2, cap), F8, kind="ExternalInput")
    wH = nc.dram_tensor("wH", (128, nP, 2, H1), F8, kind="ExternalInput")
    wL = nc.dram_tensor("wL", (128, nP, 2, H1), F8, kind="ExternalInput")
    we2 = nc.dram_tensor("we2", (128, nH1, H2), F16, kind="ExternalInput")
    wd1 = nc.dram_tensor("wd1", (128, nH2, DH), F16, kind="ExternalInput")
    wd2 = nc.dram_tensor("wd2", (128, nDH, DH), F16, kind="ExternalInput")
    wd3 = nc.dram_tensor("wd3", (128, nDH, LAT), F16, kind="ExternalInput")
    bias_all = nc.dram_tensor("bias_all", (128, NBIAS), F32, kind="ExternalInput")
    out = nc.dram_tensor("out", (LAT, cap), F16, kind="ExternalOutput")

    with tile.TileContext(nc) as tc:
        with (
            tc.tile_pool(name="wp", bufs=1) as wp,
            tc.tile_pool(name="fp", bufs=3) as fp,
            tc.tile_pool(name="ap", bufs=3) as ap,
            tc.tile_pool(name="dp", bufs=2) as dp,
            tc.tile_pool(name="ps_w", bufs=6, space="PSUM") as ps_w,
            tc.tile_pool(name="ps_o", bufs=2, space="PSUM") as ps_o,
        ):
            # ---- warmup: keep PE busy from ~1us so the p-state ramp
            # finishes during the startup DMA window
            wmt = wp.tile([128, warm_n], F16, tag="wm")
            nc.gpsimd.memset(wmt[:], 0.0)
            pwm_fl = ps_w.tile([128, TILE], F32, tag="pw")
            for _ in range(n_warm):
                nc.tensor.matmul(pwm_fl[:, :warm_n], wmt[:, :128], wmt[:],
                                 start=True, stop=True)

            # ---- weight tiles
            wH_t = wp.tile([128, nP, 2, H1], F8, tag="wH")
            wL_t = wp.tile([128, nP, 2, H1], F8, tag="wL")
            we2_t = wp.tile([128, nH1, H2], F16, tag="we2")
            wd1_t = wp.tile([128, nH2, DH], F16, tag="wd1")
            wd2_t = wp.tile([128, nDH, DH], F16, tag="wd2")
            wd3_t = wp.tile([128, nDH, LAT], F16, tag="wd3")
            bias_t = wp.tile([128, NBIAS], F32, tag="bias")

            ftb0_fl = fp.tile([128, nC, 2, TILE], F8, tag="ft")
            ftb1_fl = fp.tile([128, nC, 2, TILE], F8, tag="ft")
            ftb2_fl = fp.tile([128, nC, 2, TILE], F8, tag="ft")
            pre = {0: ftb0_fl, 1: ftb1_fl, 2: ftb2_fl}
            tn0 = tiles[0]
            # hand-ordered sync HWDGE queue: each load lands just before
            # its first-use deadline (DMA engines are a serial resource)
            for p in range(nP):
                nc.sync.dma_start(wH_t[:, p], wH[:, p])
                if p == nP - 1:
                    nc.sync.dma_start(we2_t[:], we2[:])
                nc.sync.dma_start(ftb0_fl[:, 2 * p:2 * p + 2, :, :tn0],
                                  fTp[:, 2 * p:2 * p + 2, :, :tn0])
            nc.sync.dma_start(wL_t[:], wL[:])
            nc.sync.dma_start(bias_t[:], bias_all[:])
            sl1 = slice(offs[1], offs[1] + tiles[1])
            nc.sync.dma_start(ftb1_fl[:, :, :, :tiles[1]], fTp[:, :, :, sl1])
            nc.sync.dma_start(wd1_t[:], wd1[:])
            if nt > 2:
                sl2 = slice(offs[2], offs[2] + tiles[2])
                nc.sync.dma_start(ftb2_fl[:, :, :, :tiles[2]], fTp[:, :, :, sl2])
            nc.sync.dma_start(wd2_t[:], wd2[:])
            nc.sync.dma_start(wd3_t[:], wd3[:])

            def emit_enc(t, ftb_fl):
                tn = tiles[t]
                ftb = ftb_fl[:, :, :, :tn]
                # --- L1: 3-term hi/lo fp8 DoubleRow, c-paired ---
                pws = [ps_w.tile([128, TILE], F32, tag="pw", name=f"pw1_{m}")
                       for m in range(nH1)]

                def T(m, p, hilo, wt, start=False, stop=False):
                    nc.tensor.matmul(pws[m][:, :tn],
                                     wt[:, p, :, bass.ts(m, 128)],
                                     ftb[:, 2 * p:2 * p + 2, hilo, :],
                                     start=start, stop=stop, perf_mode=DR)

                if t == 0:
                    # p-outer: consume chunk DMAs as they land
                    for p in range(nP):
                        for m in range(nH1):
                            T(m, p, 0, wH_t, start=(p == 0))
                            T(m, p, 1, wH_t)
                    for m in range(nH1):
                        for p in range(nP):
                            T(m, p, 0, wL_t, stop=(p == nP - 1))
                else:
                    # chain-major: chain m0 stops early so its relu+L2 can
                    # overlap the remaining chains
                    for m in range(nH1):
                        for p in range(nP):
                            T(m, p, 0, wH_t, start=(p == 0))
                            T(m, p, 1, wH_t)
                        for p in range(nP):
                            T(m, p, 0, wL_t, stop=(p == nP - 1))
                a1 = []
                for m in range(nH1):
                    x_fl = ap.tile([128, TILE], F16, tag=f"a1_{m}")
                    x = x_fl[:, :tn]
                    nc.scalar.activation(x[:], pws[m][:, :tn], ActF.Relu,
                                         bias=bias_t[:, OB1 + m:OB1 + m + 1],
                                         scale=L1SC)
                    a1.append(x)
                # --- L2: fp16 ---
                a2 = []
                for m in range(nH2):
                    pw_fl = ps_w.tile([128, TILE], F32, tag="pw")
                    pw = pw_fl[:, :tn]
                    for c in range(nH1):
                        nc.tensor.matmul(pw[:], we2_t[:, c, bass.ts(m, 128)],
                                         a1[c][:],
                                         start=(c == 0), stop=(c == nH1 - 1))
                    x_fl = ap.tile([128, TILE], F16, tag=f"a2_{m}")
                    x = x_fl[:, :tn]
                    if m % 2 == 0:
                        nc.vector.tensor_scalar(x[:], pw[:],
                                                bias_t[:, OB2 + m:OB2 + m + 1],
                                                0.0, Alu.add, Alu.max)
                    else:
                        nc.scalar.activation(x[:], pw[:], ActF.Relu,
                                             bias=bias_t[:, OB2 + m:OB2 + m + 1])
                    a2.append(x)
                return a2

            def emit_d1(t, h):
                tn = tiles[t]
                d1 = []
                for m in range(nDH):
                    pw_fl = ps_w.tile([128, TILE], F32, tag="pw")
                    pw = pw_fl[:, :tn]
                    for c in range(nH2):
                        nc.tensor.matmul(pw[:], wd1_t[:, c, bass.ts(m, 128)],
                                         h[c][:],
                                         start=(c == 0), stop=(c == nH2 - 1))
                    x_fl = dp.tile([128, TILE], F16, tag=f"d1_{m}")
                    x = x_fl[:, :tn]
                    if m % 2 == 1:
                        nc.vector.tensor_scalar(x[:], pw[:],
                                                bias_t[:, OD1 + m:OD1 + m + 1],
                                                0.0, Alu.add, Alu.max)
                    else:
                        nc.scalar.activation(x[:], pw[:], ActF.Relu,
                                             bias=bias_t[:, OD1 + m:OD1 + m + 1])
                    d1.append(x)
                return d1

            def emit_d2_d3_store(t, d1):
                tn = tiles[t]
                d2 = []
                for m in range(nDH):
                    pw_fl = ps_w.tile([128, TILE], F32, tag="pw")
                    pw = pw_fl[:, :tn]
                    for c in range(nDH):
                        nc.tensor.matmul(pw[:], wd2_t[:, c, bass.ts(m, 128)],
                                         d1[c][:],
                                         start=(c == 0), stop=(c == nDH - 1))
                    x_fl = dp.tile([128, TILE], F16, tag=f"d2_{m}")
                    x = x_fl[:, :tn]
                    bb = bias_t[:, OD2 + m:OD2 + m + 1]
                    if m % 2 == 0:
                        nc.vector.tensor_scalar(x[:], pw[:], bb, 0.0,
                                                Alu.add, Alu.max)
                    else:
                        nc.scalar.activation(x[:], pw[:], ActF.Relu, bias=bb)
                    d2.append(x)
                po_fl = ps_o.tile([128, TILE], F32, tag="out")
                po = po_fl[:, :tn]
                for c in range(nDH):
                    nc.tensor.matmul(po[:], wd3_t[:, c, :], d2[c][:],
                                     start=(c == 0), stop=(c == nDH - 1))
                osb_fl = ap.tile([128, TILE], F16, tag="osb")
                osb = osb_fl[:, :tn]
                nc.scalar.activation(osb[:], po[:], ActF.Identity,
                                     bias=bias_t[:, OD3:OD3 + 1])
                # stores ride the scalar HWDGE queue right behind the act
                nc.scalar.dma_start(out[:, offs[t]:offs[t] + tn], osb[:])

            # decoder runs one tile behind the encoder
            pend = None
            for t in range(nt):
                if t in pre:
                    ftb_fl = pre[t]
                else:
                    ftb_fl = fp.tile([128, nC, 2, TILE], F8, tag="ft")
                    nc.sync.dma_start(ftb_fl[:, :, :, :tiles[t]],
                                      fTp[:, :, :, offs[t]:offs[t] + tiles[t]])
                h = emit_enc(t, ftb_fl)
                if pend is not None:
                    emit_d2_d3_store(pend[0], pend[1])
                d1 = emit_d1(t, h)
                pend = (t, d1)
            emit_d2_d3_store(pend[0], pend[1])

    nc.finalize()
    return nc


def _pack_inputs(features, We1, be1, We2, be2, We3, be3,
                 Wd1, bd1, Wd2, bd2, Wd3, bd3, cat_idx, cap):
    """Dispatch rows to cores by category (expert-parallel sharding)."""
    features = np.asarray(features, np.float32)
    cat = np.asarray(cat_idx).astype(np.int64)
    order = np.argsort(cat, kind="stable")
    counts = np.bincount(cat, minlength=N_CORES)
    starts = np.zeros(N_CORES + 1, np.int64)
    np.cumsum(counts, out=starts[1:])

    def chunkcols(b):
        return np.asarray(b, np.float32).reshape(-1).reshape(-1, 128).T

    # ---- shared encoder weights (identical on every core)
    We1f = np.asarray(We1, np.float64)
    w256 = We1f * SW  # [C, H1]
    Wh = w256.astype(FP8)
    Wl = (w256 - Wh.astype(np.float64)).astype(FP8)
    Wh4 = Wh.reshape(nP, 2, 128, H1)
    Wl4 = Wl.reshape(nP, 2, 128, H1)
    wH_h = np.ascontiguousarray(Wh4.transpose(2, 0, 1, 3))
    wL_h = np.ascontiguousarray(Wl4.transpose(2, 0, 1, 3))

    def pmajor(w, n):  # [C0, C1] -> [128, n, C1] partition-major
        return np.ascontiguousarray(
            np.asarray(w, np.float16).reshape(n, 128, -1).transpose(1, 0, 2))

    we2_h = pmajor(We2, nH1)
    We3f = np.asarray(We3, np.float32)
    be3f = np.asarray(be3, np.float32)

    maps, rows_per_core = [], []
    for k in range(N_CORES):
        rows = order[starts[k]:starts[k + 1]]
        rows_per_core.append(rows)
        # features: x*16 -> hi/lo fp8, partition-major [128, nC, 2, cap]
        xT = np.zeros((C, cap), np.float32)
        xT[:, :len(rows)] = features[rows].T * SX
        xh = xT.astype(FP8)
        xl = (xT - xh.astype(np.float32)).astype(FP8)
        fTp_h = np.empty((128, nC, 2, cap), FP8)
        fTp_h[:, :, 0] = xh.reshape(nC, 128, cap).transpose(1, 0, 2)
        fTp_h[:, :, 1] = xl.reshape(nC, 128, cap).transpose(1, 0, 2)

        bias_all = np.zeros((128, NBIAS), np.float32)
        bias_all[:, OB1:OB1 + 4] = chunkcols(be1)
        bias_all[:, OB2:OB2 + 2] = chunkcols(be2)
        wd1k = np.asarray(Wd1, np.float32)[k]
        bias_all[:, OD1:OD1 + 2] = chunkcols(
            wd1k.T @ be3f + np.asarray(bd1, np.float32)[k])
        bias_all[:, OD2:OD2 + 2] = chunkcols(np.asarray(bd2, np.float32)[k])
        bias_all[:, OD3:OD3 + 1] = chunkcols(np.asarray(bd3, np.float32)[k])

        m = dict(
            fTp=fTp_h, wH=wH_h, wL=wL_h, we2=we2_h,
            wd1=pmajor(We3f @ wd1k, nH2),  # encoder L3 folded in
            wd2=pmajor(np.asarray(Wd2, np.float32)[k], nDH),
            wd3=pmajor(np.asarray(Wd3, np.float32)[k], nDH),
            bias_all=bias_all,
        )
        maps.append(m)
    return maps, rows_per_core


_NC_CACHE = {}


def _get_nc(cap=4208):
    if cap not in _NC_CACHE:
        _NC_CACHE[cap] = _build_nc(cap)
    return _NC_CACHE[cap]


def kernel(**inputs) -> np.ndarray:
    cat = np.asarray(inputs["cat_idx"]).astype(np.int64)
    counts = np.bincount(cat, minlength=N_CORES)
    cap = max(512, int(counts.max()))
    maps, rows_per_core = _pack_inputs(**inputs, cap=cap)
    nc = _get_nc(cap)
    res = bass_utils.run_bass_kernel_spmd(nc, maps, core_ids=list(range(N_CORES)))
    latent = np.zeros((B, LAT), np.float32)
    for k, r in enumerate(res.results):
        rows = rows_per_core[k]
        latent[rows] = r["out"][:, :len(rows)].T.astype(np.float32)
    return latent
